# revision 3
# baseline (speedup 1.0000x reference)
"""Trainium2 Bass kernel for nn_ActionTensorLoss.

Reference semantics (B=4096 samples, A=2048 max actions, F=8 features):
  predictions/targets: (B, A+1, F) f32; [:,0,0] carries the action count.
  count_loss  = mean((pred_counts - target_counts)^2)
  per-sample  = sum((pred_acts - targ_acts)^2 * row_mask) / max(8*count, 1)
  total = count_loss + 2 * sum(per_sample) / max(n_valid, 1)   [if n_valid>0]

Sharding: pure data-parallel over the batch dim across 8 NeuronCores
(512 samples/core). Each core reduces its shard to per-sample-lane partials
out[128, 3] = (sum count_sq, sum per_sample_mse, sum valid) accumulated over
4 groups of 128 samples; the host sums 8*128 lanes and applies the final
scalar arithmetic.

Per-core pipeline (memory-bound; ~67 MB of HBM reads per core):
  for each group (128 samples on partitions) and each of 3 column chunks
  (683 rows * 8 feats = 5464 f32 per partition):
    DMA  targ chunk, pred chunk                       (sync/HWDGE)
    DVE  diff = pred - targ            (tensor_tensor, in place)
    DVE  md = (iota < 8*(count+1)) * diff  (scalar_tensor_tensor)
    ACT  Square(md), accum_out -> per-chunk masked sum of squares
  The iota constant for chunk 0 has positions 0..7 poisoned (+1e9) so the
  header row is excluded from the action loss automatically.
"""

import os
from contextlib import ExitStack

import numpy as np

B, A, F = 4096, 2048, 8
ROWS = A + 1            # 2049 rows (header + actions)
FREE = ROWS * F         # 16392 f32 per sample
N_CORES = 8
B_CORE = B // N_CORES   # 512 samples per core
P = 128                 # partitions
GROUPS = B_CORE // P    # 4
NCH = 3                 # column chunks per group
CH_ROWS = ROWS // NCH   # 683 rows per chunk
CH = CH_ROWS * F        # 5464 elements per chunk
W_ACTION_COUNT = 1.0
W_ACTION_TENSOR = 2.0

_CACHED_NC = None


def _build():
    import concourse.bass as bass  # noqa: F401
    import concourse.tile as tile
    from concourse import bacc, mybir

    f32 = mybir.dt.float32
    Alu = mybir.AluOpType
    ActF = mybir.ActivationFunctionType

    nc = bacc.Bacc(
        "TRN2",
        target_bir_lowering=False,
        debug=False,
        num_devices=N_CORES,
    )
    pred = nc.dram_tensor(
        "predictions", [B_CORE, FREE], f32, kind="ExternalInput"
    ).ap()
    targ = nc.dram_tensor("targets", [B_CORE, FREE], f32, kind="ExternalInput").ap()
    out = nc.dram_tensor("out", [P, 3], f32, kind="ExternalOutput").ap()

    with ExitStack() as ctx:
        tc = ctx.enter_context(tile.TileContext(nc))
        io = ctx.enter_context(tc.tile_pool(name="io", bufs=3))
        consts = ctx.enter_context(tc.tile_pool(name="consts", bufs=1))
        small = ctx.enter_context(tc.tile_pool(name="small", bufs=2))
        accp = ctx.enter_context(tc.tile_pool(name="acc", bufs=1))

        # iota constants: values 0..CH-1 (exact in f32), plus a copy whose
        # first 8 entries (the header row in chunk 0) are poisoned so the
        # is_lt mask always drops them.
        iota_f = consts.tile([P, CH], f32, tag="iota_f")
        nc.gpsimd.iota(
            iota_f[:], [[1, CH]], channel_multiplier=0,
            allow_small_or_imprecise_dtypes=True,
        )
        iota_p = consts.tile([P, CH], f32, tag="iota_p")
        nc.vector.tensor_copy(iota_p[:], iota_f[:])
        nc.vector.memset(iota_p[:, 0:8], 1.0e9)

        vg = [accp.tile([P, 3], f32, tag=f"vg{g}", name=f"vg{g}") for g in range(GROUPS)]
        acc = [
            [accp.tile([P, 1], f32, tag=f"ac{g}_{c}", name=f"ac{g}_{c}") for c in range(NCH)]
            for g in range(GROUPS)
        ]

        for g in range(GROUPS):
            r0 = g * P
            c_g = small.tile([P, 1], f32, tag="c")
            e_loc = [small.tile([P, 1], f32, tag=f"e{i}", name=f"e{g}_{i}") for i in range(NCH)]
            for chx in range(NCH):
                off = chx * CH
                t_t = io.tile([P, CH], f32, tag="t")
                p_t = io.tile([P, CH], f32, tag="p")
                nc.sync.dma_start(t_t[:], targ[r0 : r0 + P, off : off + CH])
                nc.sync.dma_start(p_t[:], pred[r0 : r0 + P, off : off + CH])
                if chx == 0:
                    # count lives at [row 0, feat 0] of the targets chunk
                    nc.vector.tensor_copy(c_g[:], t_t[:, 0:1])
                    # E0 = 8*(c+1); E_i = E0 - i*CH (mask threshold, local coords)
                    nc.vector.tensor_scalar(
                        e_loc[0][:], c_g[:], 8.0, 8.0, Alu.mult, Alu.add
                    )
                    for i in range(1, NCH):
                        nc.vector.tensor_scalar(
                            e_loc[i][:], e_loc[0][:], float(i * CH), None,
                            Alu.subtract,
                        )
                # diff = pred - targ, in place over the targets tile
                nc.vector.tensor_tensor(t_t[:], p_t[:], t_t[:], Alu.subtract)
                if chx == 0:
                    # count-loss contribution: diff[0]^2
                    nc.vector.tensor_tensor(
                        vg[g][:, 0:1], t_t[:, 0:1], t_t[:, 0:1], Alu.mult
                    )
                iota_t = iota_p if chx == 0 else iota_f
                # masked diff: (iota < E) * diff, in place
                nc.vector.scalar_tensor_tensor(
                    t_t[:], iota_t[:], e_loc[chx][:], t_t[:], Alu.is_lt, Alu.mult
                )
                # sum of squares of the masked diff -> acc[g][chx]
                nc.scalar.activation(
                    p_t[:], t_t[:], ActF.Square, accum_out=acc[g][chx][:]
                )
            # group epilogue: per-sample mse and validity
            asum = small.tile([P, 1], f32, tag="asum")
            nc.vector.tensor_tensor(asum[:], acc[g][0][:], acc[g][1][:], Alu.add)
            nc.vector.tensor_tensor(asum[:], asum[:], acc[g][2][:], Alu.add)
            den = small.tile([P, 1], f32, tag="den")
            nc.vector.tensor_scalar(den[:], c_g[:], 8.0, 1.0, Alu.mult, Alu.max)
            rcp = small.tile([P, 1], f32, tag="rcp")
            nc.vector.reciprocal(rcp[:], den[:])
            nc.vector.tensor_tensor(vg[g][:, 1:2], asum[:], rcp[:], Alu.mult)
            nc.vector.tensor_scalar(
                vg[g][:, 2:3], c_g[:], 0.5, None, Alu.is_ge
            )
        # combine the 4 groups -> out[128, 3]
        v01 = accp.tile([P, 3], f32, tag="v01")
        v23 = accp.tile([P, 3], f32, tag="v23")
        nc.vector.tensor_tensor(v01[:], vg[0][:], vg[1][:], Alu.add)
        nc.vector.tensor_tensor(v23[:], vg[2][:], vg[3][:], Alu.add)
        nc.vector.tensor_tensor(v01[:], v01[:], v23[:], Alu.add)
        nc.sync.dma_start(out[:], v01[:])

    nc.compile()
    return nc


def get_nc():
    global _CACHED_NC
    if _CACHED_NC is None:
        _CACHED_NC = _build()
    return _CACHED_NC


def _make_in_maps(predictions, targets):
    p = np.asarray(predictions, dtype=np.float32).reshape(B, FREE)
    t = np.asarray(targets, dtype=np.float32).reshape(B, FREE)
    in_maps = []
    for i in range(N_CORES):
        sl = slice(i * B_CORE, (i + 1) * B_CORE)
        in_maps.append(
            {
                "predictions": np.ascontiguousarray(p[sl]),
                "targets": np.ascontiguousarray(t[sl]),
            }
        )
    return in_maps


def _combine(core_outs):
    vals = np.stack(core_outs).astype(np.float64)  # (8, 128, 3)
    csq_sum = vals[..., 0].sum()
    mse_sum = vals[..., 1].sum()
    n_valid = vals[..., 2].sum()
    count_loss = csq_sum / B
    atl = mse_sum / max(n_valid, 1.0)
    total = W_ACTION_COUNT * count_loss + (
        W_ACTION_TENSOR * atl if n_valid > 0 else 0.0
    )
    return np.array(total, dtype=np.float32)


def _ensure_ntff_hook():
    """The agent image's antenv package lacks axon_hooks, so the boot-time
    NTFF hook registration silently degrades. Recreate the module shim and
    register the ctypes hook so trace=True produces exec_time_ns."""
    import sys
    import types

    try:
        from antenv.axon_hooks import get_axon_ntff_profile_hook  # noqa: F401
        return
    except ImportError:
        pass
    mod = types.ModuleType("antenv.axon_hooks")
    _hook = [None]
    mod.set_axon_ntff_profile_hook = lambda h: _hook.__setitem__(0, h)
    mod.get_axon_ntff_profile_hook = lambda: _hook[0]
    sys.modules["antenv.axon_hooks"] = mod
    import antenv

    antenv.axon_hooks = mod
    try:
        from trn_agent_boot.trn_boot import _ntff_profile_via_ctypes

        mod.set_axon_ntff_profile_hook(
            _ntff_profile_via_ctypes("/opt/axon/libaxon_pjrt.so")
        )
    except Exception:
        pass


def run(predictions, targets, trace=False, **spmd_kwargs):
    """Run on the 8 NeuronCores; returns (output, BassKernelResults)."""
    from concourse.bass_utils import run_bass_kernel_spmd

    if trace:
        _ensure_ntff_hook()

    nc = get_nc()
    in_maps = _make_in_maps(predictions, targets)
    res = run_bass_kernel_spmd(
        nc, in_maps, core_ids=list(range(N_CORES)), trace=trace, **spmd_kwargs
    )
    out = _combine([r["out"] for r in res.results])
    return out, res


def kernel(predictions, targets):
    out, _ = run(predictions, targets, trace=False)
    return out


if __name__ == "__main__":
    np.random.seed(0)
    preds = np.random.randn(B, ROWS, F).astype(np.float32)
    targs = np.random.randn(B, ROWS, F).astype(np.float32)
    counts = np.random.randint(0, A + 1, size=B).astype(np.float32)
    targs[:, 0, 0] = counts
    print(kernel(preds, targs))
